# revision 2
# baseline (speedup 1.0000x reference)
"""AsymmetricEMA Trainium2 kernel (8 NeuronCores, Bass/Tile).

Recurrence: y_0 = x_0; y_t = a*y_{t-1} + (1-a)*x_t with a = 0.99 if
y_{t-1} > x_t else 0.5.  Equivalently (exactly):

    y_t = max(0.99*(y_{t-1}-x_t), 0.5*(y_{t-1}-x_t)) + x_t

one fused custom DVE instruction per time step.  The map contracts
(slope <= 0.99), so the time axis is split into C=4 chunks of L=1024
processed in parallel, each warmed up with W=512 extra steps started from
y := x at chunk_start - W; the warmed-up state is bit-exact (contraction +
fp32 quantization snap the state onto the true trajectory).

Sharding: batch (16) across 8 cores, 2 batches/core, pure data parallel.
Per-core layout: 128 channels on partitions; free dim = (ring column,
group=(batch, channel-block), chunk-stream).  x and y stream through SBUF
ring buffers; DMA uses the natural [time, channel] layout (contiguous 4KB
rows) with PE-transposes (idle TensorEngine) between DMA layout and
compute layout, PSUM evacuated by the Scalar engine.  Each time step is
two half-width DVE instructions (independent group halves interleave to
hide the SBUF write->read bubble between dependent steps).
"""
import numpy as np
import orjson

# ---------------------------------------------------------------------------
# container workaround: this walrus build allows ONE sync-wait per
# instruction; hoist extras onto NoOps inserted before (same engine =>
# same order => identical sync semantics).
# ---------------------------------------------------------------------------
from concourse import bass as _bass

_MAX_WAITS = 1
_orig_to_json_bytes = _bass.Bass.to_json_bytes


def _split_waits_json(data: bytes) -> bytes:
    j = orjson.loads(data)
    n = [0]
    changed = False
    for fn in j.get("functions", []):
        for bb in fn.get("blocks", []):
            out = []
            for inst in bb.get("instructions", []):
                si = inst.get("sync_info")
                if si:
                    waits = si.get("on_wait") or []
                    if len(waits) > _MAX_WAITS:
                        changed = True
                        for w in waits[:-_MAX_WAITS]:
                            n[0] += 1
                            out.append({
                                "debug": inst.get("debug", 0),
                                "engine": inst["engine"],
                                "ins": [], "outs": [],
                                "name": f"I-waitsplit-{n[0]}",
                                "opcode": "NoOp",
                                "sync_info": {"on_update": [],
                                              "on_wait": [w]},
                            })
                        si["on_wait"] = waits[-_MAX_WAITS:]
                out.append(inst)
            bb["instructions"] = out
    return orjson.dumps(j) if changed else data


def _to_json_bytes_patched(self, *a, **k):
    return _split_waits_json(_orig_to_json_bytes(self, *a, **k))


_bass.Bass.to_json_bytes = _to_json_bytes_patched

from concourse import bass, mybir, masks  # noqa: E402
from concourse.tile import TileContext  # noqa: E402
from concourse.bass_utils import run_bass_kernel_spmd  # noqa: E402

F32 = mybir.dt.float32
AF, AR = 0.99, 0.5

# ---------------------------------------------------------------------------
# fused EMA-step custom DVE op: out = max((in0-in1)*C0, (in0-in1)*C1) + in1
# ---------------------------------------------------------------------------
_EMA_OP = [None]


def _get_ema_step_op():
    if _EMA_OP[0] is not None:
        return _EMA_OP[0]
    from concourse.dve_spec import Spec, Src0, Src1, C0, C1, maxx, lower
    from concourse.dve_uop import DveOpSpec
    from concourse import dve_ops
    from concourse.dve_ops import DveOp, OPS

    def _ref(in0, in1, s0, s1, imm2):
        d = (in0 - in1).astype(np.float32)
        return (np.maximum(d * np.float32(0.99), d * np.float32(0.5))
                + in1).astype(np.float32)

    d = Src0 - Src1
    spec = Spec(body=maxx(d * C0, d * C1) + Src1, reference=_ref)
    shas = {}
    for ver in ("v3", "v4"):
        u = lower(spec, ver=ver)
        shas[ver] = DveOpSpec(name="EMA_STEP_ANT", opcode=0, uops=u,
                              rd1_en=True).sha(ver)
    op = DveOp("EMA_STEP_ANT", spec, subdim=False, uops_sha=shas)
    OPS.append(op)
    dve_ops.CUSTOM_DVE_SPECS[op.name] = op.spec
    dve_ops._SUB_OPCODE_FOR_NAME[op.name] = (
        dve_ops._CUSTOM_DVE_ROW_BASE + len(OPS) - 1)
    _EMA_OP[0] = op
    return op


# ---------------------------------------------------------------------------
# per-core SPMD program
# ---------------------------------------------------------------------------
def _build(B_PER_CORE=2, T=4096, NCH=1024, L=1024, W=512, RING=256, BLK=128):
    CBLK = NCH // 128
    G = B_PER_CORE * CBLK
    C = T // L
    assert W % BLK == 0 and L % BLK == 0 and RING % BLK == 0
    assert W <= L and RING >= 2 * BLK
    ema_op = _get_ema_step_op()

    nc = bass.Bass()
    x_ext = nc.declare_dram_parameter("x", [B_PER_CORE, T, NCH], F32,
                                      isOutput=False)
    out_ext = nc.declare_dram_parameter("out", [B_PER_CORE, T, NCH], F32,
                                        isOutput=True)

    with TileContext(nc) as tc:
        with tc.tile_pool(name="rings", bufs=1) as rpool, \
             tc.tile_pool(name="nat", bufs=8) as natpool, \
             tc.tile_pool(name="tpsum", bufs=2, space="PSUM") as psumpool, \
             tc.tile_pool(name="consts", bufs=1) as cpool:
            X = rpool.tile([128, RING, G, C], F32)
            Y = rpool.tile([128, RING, G, C], F32)
            ident = cpool.tile([128, 128], F32)
            masks.make_identity(nc, ident[:])

            def refill(ib):
                # load x columns [ib, ib+BLK) of every active stream
                r0 = ib % RING
                for b in range(B_PER_CORE):
                    for ch in range(C):
                        t0 = ch * L + ib
                        if t0 < 0 or t0 >= T or (ib < 0 and ch == 0):
                            continue
                        nat_t = natpool.tile([128, CBLK * 128], F32,
                                             tag="nat", name="nat_t")
                        nc.sync.dma_start(out=nat_t[:BLK, :],
                                          in_=x_ext[b, t0:t0 + BLK, :])
                        ps_t = psumpool.tile([128, CBLK * BLK], F32,
                                             tag="ps", name="ps_t")
                        for k in range(CBLK):
                            nc.tensor.transpose(
                                ps_t[:, k * BLK:(k + 1) * BLK],
                                nat_t[:BLK, k * 128:(k + 1) * 128],
                                ident[:BLK, :BLK])
                        g0 = b * CBLK
                        dst = X[:, r0:r0 + BLK, g0:g0 + CBLK, ch]
                        src = ps_t[:, 0:CBLK * BLK].rearrange(
                            "p (k t) -> p t k", k=CBLK)
                        nc.scalar.copy(dst, src)

            def drain_out(ib):
                # store finished y body columns [ib, ib+BLK)
                r0 = ib % RING
                for b in range(B_PER_CORE):
                    for ch in range(C):
                        po_t = psumpool.tile([128, CBLK * 128], F32,
                                             tag="po", name="po_t")
                        g0 = b * CBLK
                        for k in range(CBLK):
                            nc.tensor.transpose(
                                po_t[:BLK, k * 128:(k + 1) * 128],
                                Y[:, r0:r0 + BLK, g0 + k, ch],
                                ident[:, :])
                        st_t = natpool.tile([128, CBLK * 128], F32,
                                            tag="st", name="st_t")
                        nc.scalar.copy(st_t[:BLK, :], po_t[:BLK, :])
                        t0 = ch * L + ib
                        nc.sync.dma_start(out=out_ext[b, t0:t0 + BLK, :],
                                          in_=st_t[:BLK, :])

            def step(i, ch_lo, ch_hi, init_from_x):
                r = i % RING
                rp = (i - 1) % RING
                nsplit = 2 if G >= 2 else 1
                gs = G // nsplit
                for s in range(nsplit):
                    g0, g1 = s * gs, (s + 1) * gs
                    xcol = X[:, r, g0:g1, ch_lo:ch_hi]
                    ycol = Y[:, r, g0:g1, ch_lo:ch_hi]
                    yprev = (X[:, r, g0:g1, ch_lo:ch_hi] if init_from_x
                             else Y[:, rp, g0:g1, ch_lo:ch_hi])
                    nc.vector._custom_dve(ema_op, out=ycol, in0=yprev,
                                          in1=xcol, s0=AF, s1=AR)

            refill(-W)
            for i in range(-W, L):
                if i % BLK == 0 and i + BLK < L:
                    refill(i + BLK)
                if i == -W:
                    step(i, 1, C, init_from_x=True)
                elif i == 0:
                    step(i, 0, 1, init_from_x=True)
                    if C > 1:
                        step(i, 1, C, init_from_x=False)
                elif i < 0:
                    step(i, 1, C, init_from_x=False)
                else:
                    step(i, 0, C, init_from_x=False)
                if i >= 0 and (i + 1) % BLK == 0:
                    drain_out(i + 1 - BLK)

    mybir.codegen_inst_isa_subclasses(nc)
    return nc


_NC_CACHE = [None]


def kernel(x: np.ndarray) -> np.ndarray:
    x = np.asarray(x, dtype=np.float32)
    B, T, NCH = x.shape  # (16, 4096, 1024)
    n_cores = 8
    bpc = B // n_cores
    if _NC_CACHE[0] is None:
        _NC_CACHE[0] = _build(B_PER_CORE=bpc, T=T, NCH=NCH)
    nc = _NC_CACHE[0]
    in_maps = [{"x": np.ascontiguousarray(x[bpc * k:bpc * (k + 1)])}
               for k in range(n_cores)]
    res = run_bass_kernel_spmd(nc, in_maps, core_ids=list(range(n_cores)))
    return np.concatenate([res.results[k]["out"] for k in range(n_cores)],
                          axis=0)
